# revision 18
# baseline (speedup 1.0000x reference)
"""Trainium2 Bass kernel for the nn_EncoderBlock problem.

Full inputs in, full output out. Internally: 8-way SPMD over
(batch=2) x (query shard=4). Per block (two residual MSA blocks):
  LN(LN(x)) -> QKV projections (own 1024 tokens) -> AllGather K/V
  (bf16) within the 4-core batch group -> per-head flash-style
  attention with scores kept transposed [keys, queries] so softmax's
  exp doubles as the PSUM evacuation on the scalar engine -> output
  projection (+bias via K=1 ones-matmul) -> residual add.
"""

import sys

sys.path.insert(0, "/opt/trn_rl_repo")

import numpy as np
import ml_dtypes

import bass_rust
import concourse.bass as bass
import concourse.tile as tile
from concourse import mybir
from concourse.bass_utils import run_bass_kernel_spmd

F32 = mybir.dt.float32
F32R = mybir.dt.float32r
BF16 = mybir.dt.bfloat16
AF = mybir.ActivationFunctionType
ALU = mybir.AluOpType

P = 128
D = 384
H = 6
DK = 64
DT = D // P          # 3 D-chunks of 128
OWN = 1024           # tokens owned per core
NT = OWN // P        # 8 token tiles of 128
S = 4096             # full sequence per batch
NKC = S // P         # 32 key chunks of 128
GRP = 4              # cores per batch group
EPS = 1e-6
QT = 512             # query tile (free dim of score matmuls)
NQT = OWN // QT      # 2
KCG = 3              # key chunks per exp group (3*512 = 1536 psum cols)
VROW = H * (DK + 1)  # 390: per-kc row of V_aug (64 cols + ones col per head)

# ---------------------------------------------------------------------------
# walrus in this container caps sync-waits per instruction (1 for most,
# 0 for DMA-transpose). Hoist excess waits onto same-engine NoOps.
_WAIT_LIMIT_BY_TYPE = {"InstDmaTransposeAnt": 0}
_wfix_ctr = [0]


def _fix_sync_waits(nc):
    for f in nc.m.functions:
        for bb in f.blocks:
            out = []
            changed = False
            for ins in bb.instructions:
                si = ins.sync_info
                waits = list(si.on_wait) if si is not None else []
                limit = _WAIT_LIMIT_BY_TYPE.get(type(ins).__name__, 1)
                if len(waits) > limit:
                    keep, hoist = waits[:limit], waits[limit:]
                    for w in hoist:
                        _wfix_ctr[0] += 1
                        nop = mybir.InstNoOp(
                            name=f"WFIX-{_wfix_ctr[0]}", engine=ins.engine
                        )
                        nop.sync_info = bass_rust.SyncInfo(on_wait=[w], on_update=[])
                        out.append(nop)
                    ins.sync_info = bass_rust.SyncInfo(
                        on_wait=keep, on_update=list(si.on_update)
                    )
                    changed = True
                out.append(ins)
            if changed:
                bb.instructions = out


# ---------------------------------------------------------------------------
def _build_block(nc, tc, pools, consts, x_src, x_dst, ra_bc, rb_bc, ccin, ccout,
                 consts_extra=None):
    """One residual MSA block: x_dst = x_src + MSA(LN(LN(x_src)))."""
    work = pools["work"]
    psA = pools["psA"]      # 2 x [128,1536] f32 = 6 psum banks
    psB = pools["psB"]      # 2 x [128,512]  f32 = 2 psum banks
    ste_pool = pools["ste"]

    (wqT, wkT, wvT, woT, bq_col, bk_col, bv_bc, bo_row, a0_bc, b0_bc,
     ones_t, kt_full, v_aug, qt_sb, kt_sh, v_sh, zb, zT, y1, ot) = consts

    # ---- LN(LN(x)) over own tokens; zb (bf16) = final normalized z ----
    for (src, m_ra, m_rb, dst) in ((x_src, ra_bc, rb_bc, y1),
                                   (y1, a0_bc, b0_bc, zb)):
        stats = work.tile([P, NT, 6], F32, tag="ln_stats")
        mv = work.tile([P, NT, 2], F32, tag="ln_mv")
        for n in range(NT):
            nc.vector.bn_stats(out=stats[:, n, :], in_=src[:, n, :])
            nc.vector.bn_aggr(out=mv[:, n, :], in_=stats[:, n, :])
        # rstd = 1 / (sqrt(var * N/(N-1)) + eps), batched over the 8 tiles
        rs = work.tile([P, NT], F32, tag="ln_rs")
        nc.scalar.activation(out=rs[:, :], in_=mv[:, :, 1], func=AF.Sqrt,
                             scale=float(D) / float(D - 1))
        nc.vector.tensor_scalar_add(out=rs[:, :], in0=rs[:, :], scalar1=EPS)
        nc.vector.reciprocal(out=rs[:, :], in_=rs[:, :])
        for n in range(NT):
            t = work.tile([P, D], F32, tag="ln_t")
            nc.vector.tensor_scalar(
                out=t[:, :], in0=src[:, n, :],
                scalar1=mv[:, n, 0:1], scalar2=rs[:, n:n + 1],
                op0=ALU.subtract, op1=ALU.mult)
            nc.vector.tensor_mul(out=t[:, :], in0=t[:, :], in1=m_ra[:, :])
            nc.vector.tensor_add(out=dst[:, n, :], in0=t[:, :], in1=m_rb[:, :])

    # ---- transpose zb [tok, D] -> zT [D, tok] (bf16, PE transpose) ----
    ident = consts_extra["ident"]
    for n in range(NT):
        for dt_ in range(DT):
            tp = psB.tile([P, P], BF16, tag="acc", name="tp")
            nc.tensor.transpose(
                out=tp[:, 0:P],
                in_=zb[:, n, dt_ * P:(dt_ + 1) * P],
                identity=ident[:, :])
            nc.vector.tensor_copy(
                out=zT[:, dt_, n * P:(n + 1) * P], in_=tp[:, 0:P])

    # ---- projections: Qt/Kt (transposed layout) and V (natural) ----
    for (wT, b_col, dst) in ((wqT, bq_col, qt_sb), (wkT, bk_col, kt_sh)):
        for dt_ in range(DT):
            for ntk in range(NQT):
                ps = psB.tile([P, QT], F32, tag="acc")
                for ki in range(DT):
                    nc.tensor.matmul(
                        ps[:, :],
                        lhsT=wT[:, ki, dt_ * P:(dt_ + 1) * P],
                        rhs=zT[:, ki, ntk * QT:(ntk + 1) * QT],
                        start=(ki == 0), stop=(ki == DT - 1))
                nc.vector.tensor_scalar(
                    out=dst[:, dt_, ntk * QT:(ntk + 1) * QT], in0=ps[:, :],
                    scalar1=b_col[:, dt_:dt_ + 1], scalar2=None, op0=ALU.add)
    for n in range(NT):
        ps = psB.tile([P, QT], F32, tag="acc")
        for ki in range(DT):
            nc.tensor.matmul(
                ps[:, :D],
                lhsT=zT[:, ki, n * P:(n + 1) * P],
                rhs=wvT[:, ki, :],
                start=(ki == 0), stop=(ki == DT - 1))
        # write into the ones-augmented 65-stride layout (ones persist at j=64)
        nc.vector.tensor_tensor(
            out=v_sh[:, n, :].rearrange(
                "p (h j) -> p h j", h=H, j=DK + 1)[:, :, 0:DK],
            in0=ps[:, :D].rearrange("p (h j) -> p h j", h=H, j=DK),
            in1=bv_bc[:, :].rearrange("p (h j) -> p h j", h=H, j=DK),
            op=ALU.add)

    # ---- stage shards to DRAM, AllGather within batch group, load back ----
    # ccin layout: [K section: (d, t) 384*1024][V section: (t, r) 1024*390]
    KSEC = D * 1024
    VSEC = 1024 * VROW
    nc.sync.dma_start(
        out=ccin[0:KSEC].rearrange("(d p t) -> p d t", d=DT, p=P),
        in_=kt_sh[:, :, :])
    nc.sync.dma_start(
        out=ccin[KSEC:KSEC + VSEC].rearrange("(n p r) -> p n r", n=NT, p=P),
        in_=v_sh[:, :, :])
    nc.gpsimd.collective_compute(
        "AllGather", ALU.bypass,
        ins=[ccin[:]], outs=[ccout[:]],
        replica_groups=[[0, 1, 2, 3], [4, 5, 6, 7]])
    for g in range(GRP):
        # K: dest [p, dt, g*1024+t] <- src[g][(dt*128+p)*1024 + t]
        nc.sync.dma_start(
            out=kt_full[:, :, g * 1024:(g + 1) * 1024],
            in_=ccout[g, 0:KSEC].rearrange("(d p t) -> p d t", d=DT, p=P))
        # V(+ones): dest [p, (g,k8), 390] <- src[g][V + ((k8*128+p)*390 + r)]
        nc.sync.dma_start(
            out=v_aug[:, :].rearrange("p (g k r) -> p g k r",
                                      g=GRP, k=8)[:, g],
            in_=ccout[g, KSEC:KSEC + VSEC].rearrange(
                "(k p r) -> p k r", k=8, p=P))

    # ---- attention ----
    vaug3 = v_aug[:, :].rearrange("p (k r) -> p k r", k=NKC)
    n_groups = (NKC + KCG - 1) // KCG
    for ntk in range(NQT):
        q0 = ntk * QT
        for hp in range(DT):
            pv = [psB.tile([P, QT], F32, tag="acc", name=f"pv{_h}")
                  for _h in range(2)]
            for g in range(n_groups):
                kcs = list(range(g * KCG, min(NKC, (g + 1) * KCG)))
                w = len(kcs) * QT
                for half in range(2):      # head pair: rows 0-63 / 64-127
                    lo = half * DK
                    st = psA.tile([P, KCG * QT], F32, tag="st")
                    for j, kc in enumerate(kcs):
                        nc.tensor.matmul(
                            st[:, j * QT:(j + 1) * QT],
                            lhsT=kt_full[lo:lo + DK, hp, kc * P:(kc + 1) * P],
                            rhs=qt_sb[lo:lo + DK, hp, q0:q0 + QT],
                            start=True, stop=True)
                    ste = ste_pool.tile([P, KCG * QT], BF16, tag="ste")
                    nc.scalar.activation(out=ste[:, :w], in_=st[:, :w],
                                         func=AF.Exp, scale=1.0 / 8.0)
                    h = 2 * hp + half
                    for j, kc in enumerate(kcs):
                        nc.tensor.matmul(
                            pv[half][0:DK + 1, :],
                            lhsT=vaug3[:, kc, h * (DK + 1):(h + 1) * (DK + 1)],
                            rhs=ste[:, j * QT:(j + 1) * QT],
                            start=(kc == 0), stop=(kc == NKC - 1),
                            skip_group_check=True)
            for half in range(2):
                lo = half * DK
                r_row = work.tile([1, QT], F32R, tag="r_row")
                with nc.allow_low_precision(
                        reason="f32r broadcast of softmax denom"):
                    nc.vector.reciprocal(
                        out=r_row[:, :], in_=pv[half][DK:DK + 1, :])
                r_bc = psA.tile([P, KCG * QT], F32, tag="st")
                nc.tensor.matmul(
                    r_bc[0:DK, 0:QT],
                    lhsT=ones_t[0:1, 0:DK],
                    rhs=r_row[0:1, :],
                    start=True, stop=True)
                r_sb = work.tile([DK, QT], F32, tag="r_sb")
                nc.vector.tensor_copy(out=r_sb[:, :], in_=r_bc[0:DK, 0:QT])
                nc.vector.tensor_tensor(
                    out=ot[lo:lo + DK, hp, q0:q0 + QT],
                    in0=pv[half][0:DK, :], in1=r_sb[:, :], op=ALU.mult)

    # ---- output projection + bias + residual ----
    for n in range(NT):
        ps = psB.tile([P, QT], F32, tag="acc")
        for ki in range(DT):
            nc.tensor.matmul(
                ps[:, :D],
                lhsT=ot[:, ki, n * P:(n + 1) * P],
                rhs=woT[:, ki, :],
                start=(ki == 0), stop=False)
        nc.tensor.matmul(
            ps[:, :D],
            lhsT=ones_t[0:1, 0:P],
            rhs=bo_row[0:1, :],
            start=False, stop=True, skip_group_check=True)
        nc.vector.tensor_tensor(
            out=x_dst[:, n, :], in0=ps[:, :D], in1=x_src[:, n, :], op=ALU.add)


def _build_program(debug=False):
    nc = bass.Bass("TRN2", target_bir_lowering=False, debug=False, num_devices=8)

    di = {}
    di["xs"] = nc.dram_tensor("xs", [OWN, D], F32, kind="ExternalInput")
    for w in ("wqT", "wkT", "wvT", "woT"):
        di[w] = nc.dram_tensor(w, [D, D], BF16, kind="ExternalInput")
    di["bq_col"] = nc.dram_tensor("bq_col", [P, DT], F32, kind="ExternalInput")
    di["bk_col"] = nc.dram_tensor("bk_col", [P, DT], F32, kind="ExternalInput")
    di["bv_bc"] = nc.dram_tensor("bv_bc", [P, D], F32, kind="ExternalInput")
    di["bo_row"] = nc.dram_tensor("bo_row", [1, D], F32R, kind="ExternalInput")
    for w in ("ra0_bc", "rb0_bc", "ra1_bc", "rb1_bc", "a0_bc", "b0_bc"):
        di[w] = nc.dram_tensor(w, [P, D], F32, kind="ExternalInput")
    di["ones_in"] = nc.dram_tensor("ones_in", [1, P], F32R, kind="ExternalInput")
    di["ident_in"] = nc.dram_tensor("ident_in", [P, P], BF16, kind="ExternalInput")
    out_d = nc.dram_tensor("out", [OWN, D], F32, kind="ExternalOutput")
    dbg = {}
    if debug:
        dbg["zb"] = nc.dram_tensor("dbg_zb", [P, NT, D], BF16, kind="ExternalOutput")
        dbg["zT"] = nc.dram_tensor("dbg_zT", [P, DT, OWN], BF16, kind="ExternalOutput")
        dbg["qt"] = nc.dram_tensor("dbg_qt", [P, DT, OWN], BF16, kind="ExternalOutput")
        dbg["ktf"] = nc.dram_tensor("dbg_ktf", [P, DT, S], BF16, kind="ExternalOutput")
        dbg["vaug"] = nc.dram_tensor("dbg_vaug", [P, NKC * VROW], BF16, kind="ExternalOutput")
        dbg["ot"] = nc.dram_tensor("dbg_ot", [P, DT, OWN], BF16, kind="ExternalOutput")
        dbg["x2"] = nc.dram_tensor("dbg_x2", [P, NT, D], F32, kind="ExternalOutput")

    with tile.TileContext(nc) as tc:
        with tc.tile_pool(name="const", bufs=1) as const, \
             tc.tile_pool(name="work", bufs=3) as work, \
             tc.tile_pool(name="ste", bufs=6) as ste_pool, \
             tc.tile_pool(name="psA", bufs=2, space="PSUM") as psA, \
             tc.tile_pool(name="psB", bufs=2, space="PSUM") as psB, \
             tc.tile_pool(name="dram", bufs=1, space="DRAM") as dram:

            pools = {"work": work, "psA": psA, "psB": psB, "ste": ste_pool}

            # persistent sbuf tensors
            wqT = const.tile([P, DT, D], BF16)
            wkT = const.tile([P, DT, D], BF16)
            wvT = const.tile([P, DT, D], BF16)
            woT = const.tile([P, DT, D], BF16)
            bq_col = const.tile([P, DT], F32)
            bk_col = const.tile([P, DT], F32)
            bv_bc = const.tile([P, D], F32)
            bo_row = const.tile([1, D], F32R)
            ln_bc = {}
            for wname in ("ra0_bc", "rb0_bc", "ra1_bc", "rb1_bc",
                          "a0_bc", "b0_bc"):
                ln_bc[wname] = const.tile([P, D], F32, tag=wname, name=wname)
            ones_t = const.tile([1, P], F32R)
            x_own = const.tile([P, NT, D], F32)
            x2 = const.tile([P, NT, D], F32)
            x3 = const.tile([P, NT, D], F32)
            kt_full = const.tile([P, DT, S], BF16)
            v_aug = const.tile([P, NKC * VROW], BF16)
            qt_sb = const.tile([P, DT, OWN], BF16)
            kt_sh = const.tile([P, DT, OWN], BF16)
            v_sh = const.tile([P, NT, VROW], BF16)
            zb = const.tile([P, NT, D], BF16)
            zT = const.tile([P, DT, OWN], BF16)
            y1 = const.tile([P, NT, D], F32)
            ot = const.tile([P, DT, OWN], BF16)

            ccsz = D * 1024 + 1024 * VROW
            ccin = [dram.tile([ccsz], BF16, tag=f"ccin{i}", name=f"ccin{i}")
                    for i in range(2)]
            ccout = [dram.tile([GRP, ccsz], BF16,
                               tag=f"ccout{i}", name=f"ccout{i}")
                     for i in range(2)]

            # loads
            for wname, t in (("wqT", wqT), ("wkT", wkT), ("wvT", wvT),
                             ("woT", woT)):
                nc.sync.dma_start(
                    out=t[:, :, :],
                    in_=di[wname][:].rearrange("(d p) e -> p d e", p=P))
            for wname, t in (("bq_col", bq_col), ("bk_col", bk_col),
                             ("bv_bc", bv_bc), ("bo_row", bo_row)):
                nc.sync.dma_start(out=t[:, :], in_=di[wname][:])
            for wname in ln_bc:
                nc.sync.dma_start(out=ln_bc[wname][:, :], in_=di[wname][:])
            nc.sync.dma_start(
                out=x_own[:, :, :],
                in_=di["xs"][:].rearrange("(n p) e -> p n e", p=P))
            nc.sync.dma_start(out=ones_t[:, :], in_=di["ones_in"][:])
            ident = const.tile([P, P], BF16)
            nc.sync.dma_start(out=ident[:, :], in_=di["ident_in"][:])
            consts_extra = {"ident": ident}
            nc.vector.memset(v_sh[:, :, :], 1.0)

            consts1 = (wqT, wkT, wvT, woT, bq_col, bk_col, bv_bc, bo_row,
                       ln_bc["a0_bc"], ln_bc["b0_bc"], ones_t, kt_full, v_aug,
                       qt_sb, kt_sh, v_sh, zb, zT, y1, ot)

            _build_block(nc, tc, pools, consts1, x_own, x2,
                         ln_bc["ra0_bc"], ln_bc["rb0_bc"], ccin[0], ccout[0],
                         consts_extra=consts_extra)
            if debug:
                for name, t in (("zb", zb), ("zT", zT), ("qt", qt_sb),
                                ("ktf", kt_full), ("vaug", v_aug),
                                ("ot", ot), ("x2", x2)):
                    nc.sync.dma_start(out=dbg[name][:], in_=t[:])
            _build_block(nc, tc, pools, consts1, x2, x3,
                         ln_bc["ra1_bc"], ln_bc["rb1_bc"], ccin[1], ccout[1],
                         consts_extra=consts_extra)

            nc.sync.dma_start(
                out=out_d[:].rearrange("(n p) e -> p n e", p=P),
                in_=x3[:, :, :])

    _fix_sync_waits(nc)
    return nc


_NC_CACHE = None


def _get_nc():
    global _NC_CACHE
    if _NC_CACHE is None:
        _NC_CACHE = _build_program()
    return _NC_CACHE


def _prep_inputs(x, a0, b0, ra0, rb0, ra1, rb1,
                 wq, bq, wk, bk, wv, bv, wo, bo):
    bf = ml_dtypes.bfloat16
    base = {
        "wqT": np.ascontiguousarray(np.asarray(wq, np.float32).T).astype(bf),
        "wkT": np.ascontiguousarray(np.asarray(wk, np.float32).T).astype(bf),
        "wvT": np.ascontiguousarray(np.asarray(wv, np.float32).T).astype(bf),
        "woT": np.ascontiguousarray(np.asarray(wo, np.float32).T).astype(bf),
        "bq_col": np.ascontiguousarray(
            np.asarray(bq, np.float32).reshape(DT, P).T),
        "bk_col": np.ascontiguousarray(
            np.asarray(bk, np.float32).reshape(DT, P).T),
        "bv_bc": np.ascontiguousarray(
            np.broadcast_to(np.asarray(bv, np.float32), (P, D))),
        "bo_row": np.asarray(bo, np.float32).reshape(1, D).copy(),
        "ra0_bc": np.ascontiguousarray(
            np.broadcast_to(np.asarray(ra0, np.float32), (P, D))),
        "rb0_bc": np.ascontiguousarray(
            np.broadcast_to(np.asarray(rb0, np.float32), (P, D))),
        "ra1_bc": np.ascontiguousarray(
            np.broadcast_to(np.asarray(ra1, np.float32), (P, D))),
        "rb1_bc": np.ascontiguousarray(
            np.broadcast_to(np.asarray(rb1, np.float32), (P, D))),
        "a0_bc": np.ascontiguousarray(
            np.broadcast_to(np.asarray(a0, np.float32), (P, D))),
        "b0_bc": np.ascontiguousarray(
            np.broadcast_to(np.asarray(b0, np.float32), (P, D))),
        "ones_in": np.ones((1, P), np.float32),
        "ident_in": np.eye(P, dtype=np.float32).astype(ml_dtypes.bfloat16),
    }
    x = np.asarray(x, np.float32)
    in_maps = []
    for c in range(8):
        b, q0 = c // GRP, (c % GRP) * OWN
        m = dict(base)
        m["xs"] = np.ascontiguousarray(x[b, q0:q0 + OWN, :])
        in_maps.append(m)
    return in_maps


def kernel(**inputs):
    nc = _get_nc()
    in_maps = _prep_inputs(**inputs)
    res = run_bass_kernel_spmd(nc, in_maps, list(range(8)))
    B = inputs["x"].shape[0]
    out = np.empty((B, S, D), np.float32)
    for c in range(8):
        b, q0 = c // GRP, (c % GRP) * OWN
        out[b, q0:q0 + OWN, :] = res.results[c]["out"]
    return out


if __name__ == "__main__":
    rng = np.random.default_rng(0)
    ins = {
        "x": rng.standard_normal((2, S, D)).astype(np.float32),
        "a0": np.ones(D, np.float32), "b0": np.zeros(D, np.float32),
        "ra0": np.ones(D, np.float32), "rb0": np.zeros(D, np.float32),
        "ra1": np.ones(D, np.float32), "rb1": np.zeros(D, np.float32),
        "wq": (rng.standard_normal((D, D)) * 0.02).astype(np.float32),
        "bq": np.zeros(D, np.float32),
        "wk": (rng.standard_normal((D, D)) * 0.02).astype(np.float32),
        "bk": np.zeros(D, np.float32),
        "wv": (rng.standard_normal((D, D)) * 0.02).astype(np.float32),
        "bv": np.zeros(D, np.float32),
        "wo": (rng.standard_normal((D, D)) * 0.02).astype(np.float32),
        "bo": np.zeros(D, np.float32),
    }
    out = kernel(**ins)
    print("kernel ran, out shape", out.shape, out.dtype)


# revision 23
# speedup vs baseline: 18.7124x; 18.7124x over previous
"""Trainium2 Bass kernel for the nn_EncoderBlock problem.

Full inputs in, full output out. 8-way SPMD: cores 0-3 handle batch 0,
cores 4-7 batch 1. No cross-core communication (collectives are
pathologically slow and flaky under this container's runtime), so
block 1 is computed redundantly for the whole batch on each core (its
output x2 feeds block-2 K/V for every token), while block-2 attention
and output rows are 4-way query-sharded within the batch group.

All 8 cores run the SAME program: attention is permutation-invariant
over keys, so the host rotates each core's token order by its query
offset -- "queries 0..1023" on the device are exactly the core's own
output shard, while the key set stays complete.

Per block: LN(LN(x)) -> QKV projections -> per-head attention with
scores kept transposed [keys, queries] so softmax's exp doubles as the
PSUM->SBUF evacuation on the scalar engine (no max-subtraction needed:
|scores| < 2), P*V via a ones-augmented V (M=65) so the softmax
denominator falls out of the same matmul, normalization via a K=1
outer-product broadcast matmul, output projection with bias folded in
as a K=1 matmul, residual add. bf16 matmul operands, f32 accumulation,
f32 residual stream. Score matmuls are row-packed two heads at a time
(K=64 pairs on array rows 0-63/64-127).
"""

import sys

sys.path.insert(0, "/opt/trn_rl_repo")

import numpy as np
import ml_dtypes

import bass_rust
import concourse.bass as bass
import concourse.tile as tile
from concourse import mybir
from concourse.bass_utils import run_bass_kernel_spmd

F32 = mybir.dt.float32
F32R = mybir.dt.float32r
BF16 = mybir.dt.bfloat16
AF = mybir.ActivationFunctionType
ALU = mybir.AluOpType

P = 128
D = 384
H = 6
DK = 64
DT = D // P          # 3 D-chunks of 128
S = 4096             # full sequence per batch
NTS = S // P         # 32 token tiles of 128
NKC = S // P         # 32 key chunks of 128
OWN = 1024           # block-2 query tokens owned per core
GRP = 4              # cores per batch group
EPS = 1e-6
QT = 512             # query tile (free dim of score matmuls)
KCG = 3              # key chunks per exp group (3*512 = 1536 psum cols)
VROW = H * (DK + 1)  # 390: per-kc row of V_aug (64 data cols + ones col/head)

# ---------------------------------------------------------------------------
# walrus in this container caps sync-waits per instruction (1 for most,
# 0 for DMA-transpose). Hoist excess waits onto same-engine NoOps.
_WAIT_LIMIT_BY_TYPE = {"InstDmaTransposeAnt": 0}
_wfix_ctr = [0]


def _fix_sync_waits(nc):
    for f in nc.m.functions:
        for bb in f.blocks:
            out = []
            changed = False
            for ins in bb.instructions:
                si = ins.sync_info
                waits = list(si.on_wait) if si is not None else []
                limit = _WAIT_LIMIT_BY_TYPE.get(type(ins).__name__, 1)
                if len(waits) > limit:
                    keep, hoist = waits[:limit], waits[limit:]
                    for w in hoist:
                        _wfix_ctr[0] += 1
                        nop = mybir.InstNoOp(
                            name=f"WFIX-{_wfix_ctr[0]}", engine=ins.engine
                        )
                        nop.sync_info = bass_rust.SyncInfo(on_wait=[w], on_update=[])
                        out.append(nop)
                    ins.sync_info = bass_rust.SyncInfo(
                        on_wait=keep, on_update=list(si.on_update)
                    )
                    changed = True
                out.append(ins)
            if changed:
                bb.instructions = out


def _rows(dram_ap, row0, nrows):
    """[nrows, D] f32 rows of a [*, D] DRAM tensor as a DMA AP."""
    return bass.AP(tensor=dram_ap.tensor,
                   offset=dram_ap.offset + row0 * D,
                   ap=[[D, nrows], [1, D]])


# ---------------------------------------------------------------------------
def _build_block(nc, pools, C, x_src_d, out_d, nq, blk):
    """One residual MSA block.

    x_src_d: DRAM AP [S, D] f32 -- input rows, full batch
    out_d:   DRAM AP [>=nq, D] f32 -- gets x_src[0:nq] + MSA(...)[0:nq]
    nq:      number of query rows (from token 0) to compute/output
    """
    work, psA, psB, ste_pool, otp = (pools[k] for k in
                                     ("work", "psA", "psB", "ste", "ot"))
    nqt = nq // QT

    # ---- LN(LN(x)) for all S tokens -> zT bf16 [D, S], per-tile ----
    zT = C["zT"]

    def _ln_pass(src_ap, m_ra, m_rb, dst_ap, uid):
        mv = work.tile([P, 6 + 2], F32, tag="ln_mv", name=f"mv_{uid}")
        nc.vector.bn_stats(out=mv[:, 0:6], in_=src_ap)
        nc.vector.bn_aggr(out=mv[:, 6:8], in_=mv[:, 0:6])
        r = work.tile([P, 1], F32, tag="ln_r", name=f"r_{uid}")
        nc.scalar.activation(out=r[:, :], in_=mv[:, 7:8], func=AF.Sqrt,
                             scale=float(D) / float(D - 1))
        nc.vector.tensor_scalar_add(out=r[:, :], in0=r[:, :], scalar1=EPS)
        nc.vector.reciprocal(out=r[:, :], in_=r[:, :])
        t = work.tile([P, D], F32, tag="ln_t", name=f"t_{uid}")
        nc.vector.tensor_scalar(
            out=t[:, :], in0=src_ap,
            scalar1=mv[:, 6:7], scalar2=r[:, 0:1],
            op0=ALU.subtract, op1=ALU.mult)
        nc.vector.tensor_mul(out=t[:, :], in0=t[:, :], in1=m_ra[:, :])
        nc.vector.tensor_add(out=dst_ap, in0=t[:, :], in1=m_rb[:, :])

    for n in range(NTS):
        xt = work.tile([P, D], F32, tag="x_ln", name=f"xln{blk}_{n}")
        nc.sync.dma_start(out=xt[:, :], in_=_rows(x_src_d, n * P, P))
        yt = work.tile([P, D], F32, tag="y1", name=f"y1_{blk}_{n}")
        _ln_pass(xt[:, :], C["ra_bc"], C["rb_bc"], yt[:, :], f"{blk}_{n}a")
        zb = work.tile([P, D], BF16, tag="zb", name=f"zb_{blk}_{n}")
        _ln_pass(yt[:, :], C["a0_bc"], C["b0_bc"], zb[:, :], f"{blk}_{n}b")
        for dt_ in range(DT):
            tp = psB.tile([P, P], BF16, tag="acc", name=f"tp{blk}_{n}_{dt_}")
            nc.tensor.transpose(out=tp[:, 0:P],
                                in_=zb[:, dt_ * P:(dt_ + 1) * P],
                                identity=C["ident"][:, :])
            nc.vector.tensor_copy(out=zT[:, dt_, n * P:(n + 1) * P],
                                  in_=tp[:, 0:P])

    # ---- projections: Qt (first nq tokens), Kt + V_aug (all S) ----
    qt_sb, kt_sb, v_aug = C["qt"], C["kt"], C["v_aug"]
    for (wT, b_col, dst, ncols) in ((C["wqT"], C["bq_col"], qt_sb, nq),
                                    (C["wkT"], C["bk_col"], kt_sb, S)):
        for dt_ in range(DT):
            for ntk in range(ncols // QT):
                ps = psB.tile([P, QT], F32, tag="acc",
                              name=f"p{blk}_{dt_}_{ntk}_{ncols}")
                for ki in range(DT):
                    nc.tensor.matmul(
                        ps[:, :],
                        lhsT=wT[:, ki, dt_ * P:(dt_ + 1) * P],
                        rhs=zT[:, ki, ntk * QT:(ntk + 1) * QT],
                        start=(ki == 0), stop=(ki == DT - 1))
                nc.vector.tensor_scalar(
                    out=dst[:, dt_, ntk * QT:(ntk + 1) * QT], in0=ps[:, :],
                    scalar1=b_col[:, dt_:dt_ + 1], scalar2=None, op0=ALU.add)
    for n in range(NTS):
        ps = psB.tile([P, QT], F32, tag="acc", name=f"v{blk}_{n}")
        for ki in range(DT):
            nc.tensor.matmul(
                ps[:, :D],
                lhsT=zT[:, ki, n * P:(n + 1) * P],
                rhs=C["wvT"][:, ki, :],
                start=(ki == 0), stop=(ki == DT - 1))
        # ones-augmented 65-stride layout; ones at j=64 persist from memset
        nc.vector.tensor_tensor(
            out=v_aug[:, n, :].rearrange(
                "p (h j) -> p h j", h=H, j=DK + 1)[:, :, 0:DK],
            in0=ps[:, :D].rearrange("p (h j) -> p h j", h=H, j=DK),
            in1=C["bv_bc"][:, :].rearrange("p (h j) -> p h j", h=H, j=DK),
            op=ALU.add)

    # ---- attention (queries 0..nq) + per-qt output projection ----
    n_groups = (NKC + KCG - 1) // KCG
    for ntk in range(nqt):
        ot = otp.tile([P, DT, QT], BF16, tag="ot", name=f"ot{blk}_{ntk}")
        for hp in range(DT):
            pv = [psB.tile([P, QT], F32, tag="acc",
                           name=f"pv{blk}_{ntk}_{hp}_{i}") for i in range(2)]
            for g in range(n_groups):
                kcs = list(range(g * KCG, min(NKC, (g + 1) * KCG)))
                w = len(kcs) * QT
                for half in range(2):   # head pair on partitions 0-63/64-127
                    lo = half * DK
                    st = psA.tile([P, KCG * QT], F32, tag="st",
                                  name=f"st{blk}_{ntk}_{hp}_{g}_{half}")
                    for j, kc in enumerate(kcs):
                        nc.tensor.matmul(
                            st[:, j * QT:(j + 1) * QT],
                            lhsT=kt_sb[lo:lo + DK, hp, kc * P:(kc + 1) * P],
                            rhs=qt_sb[lo:lo + DK, hp,
                                      ntk * QT:(ntk + 1) * QT],
                            start=True, stop=True)
                    ste = ste_pool.tile([P, KCG * QT], BF16, tag="ste",
                                        name=f"se{blk}_{ntk}_{hp}_{g}_{half}")
                    nc.scalar.activation(out=ste[:, :w], in_=st[:, :w],
                                         func=AF.Exp, scale=1.0 / 8.0)
                    h = 2 * hp + half
                    for j, kc in enumerate(kcs):
                        nc.tensor.matmul(
                            pv[half][0:DK + 1, :],
                            lhsT=v_aug[:, kc,
                                       h * (DK + 1):(h + 1) * (DK + 1)],
                            rhs=ste[:, j * QT:(j + 1) * QT],
                            start=(kc == 0), stop=(kc == NKC - 1),
                            skip_group_check=True)
            for half in range(2):
                lo = half * DK
                r_row = work.tile([1, QT], F32R, tag="r_row",
                                  name=f"rr{blk}_{ntk}_{hp}_{half}")
                with nc.allow_low_precision(
                        reason="f32r broadcast of softmax denom"):
                    nc.vector.reciprocal(
                        out=r_row[:, :], in_=pv[half][DK:DK + 1, :])
                r_bc = psA.tile([P, KCG * QT], F32, tag="st",
                                name=f"rb{blk}_{ntk}_{hp}_{half}")
                nc.tensor.matmul(
                    r_bc[0:DK, 0:QT],
                    lhsT=C["ones"][0:1, 0:DK],
                    rhs=r_row[0:1, :],
                    start=True, stop=True)
                r_sb = work.tile([DK, QT], F32, tag="r_sb",
                                 name=f"rs{blk}_{ntk}_{hp}_{half}")
                nc.vector.tensor_copy(out=r_sb[:, :], in_=r_bc[0:DK, 0:QT])
                nc.vector.tensor_tensor(
                    out=ot[lo:lo + DK, hp, :],
                    in0=pv[half][0:DK, :], in1=r_sb[:, :], op=ALU.mult)
        # output projection + bias + residual for this query tile
        for c4 in range(QT // P):
            tok = ntk * QT + c4 * P
            ps = psB.tile([P, QT], F32, tag="acc",
                          name=f"o{blk}_{ntk}_{c4}")
            for ki in range(DT):
                nc.tensor.matmul(
                    ps[:, :D],
                    lhsT=ot[:, ki, c4 * P:(c4 + 1) * P],
                    rhs=C["woT"][:, ki, :],
                    start=(ki == 0), stop=False)
            nc.tensor.matmul(
                ps[:, :D],
                lhsT=C["ones"][0:1, 0:P],
                rhs=C["bo_row"][0:1, :],
                start=False, stop=True, skip_group_check=True)
            xr = work.tile([P, D], F32, tag="x_res",
                           name=f"xr{blk}_{ntk}_{c4}")
            nc.sync.dma_start(out=xr[:, :], in_=_rows(x_src_d, tok, P))
            xo = work.tile([P, D], F32, tag="x_out",
                           name=f"xo{blk}_{ntk}_{c4}")
            nc.vector.tensor_tensor(
                out=xo[:, :], in0=ps[:, :D], in1=xr[:, :], op=ALU.add)
            nc.sync.dma_start(out=_rows(out_d, tok, P), in_=xo[:, :])


def _build_program():
    nc = bass.Bass("TRN2", target_bir_lowering=False, debug=False,
                   num_devices=8)

    di = {}
    di["xs"] = nc.dram_tensor("xs", [S, D], F32, kind="ExternalInput")
    for w in ("wqT", "wkT", "wvT", "woT"):
        di[w] = nc.dram_tensor(w, [D, D], BF16, kind="ExternalInput")
    di["bq_col"] = nc.dram_tensor("bq_col", [P, DT], F32, kind="ExternalInput")
    di["bk_col"] = nc.dram_tensor("bk_col", [P, DT], F32, kind="ExternalInput")
    di["bv_bc"] = nc.dram_tensor("bv_bc", [P, D], F32, kind="ExternalInput")
    di["bo_row"] = nc.dram_tensor("bo_row", [1, D], F32R, kind="ExternalInput")
    for w in ("ra0_bc", "rb0_bc", "ra1_bc", "rb1_bc", "a0_bc", "b0_bc"):
        di[w] = nc.dram_tensor(w, [P, D], F32, kind="ExternalInput")
    di["ones_in"] = nc.dram_tensor("ones_in", [1, P], F32R,
                                   kind="ExternalInput")
    di["ident_in"] = nc.dram_tensor("ident_in", [P, P], BF16,
                                    kind="ExternalInput")
    out_d = nc.dram_tensor("out", [OWN, D], F32, kind="ExternalOutput")
    x2_d = nc.dram_tensor("x2buf", [S, D], F32)   # internal

    with tile.TileContext(nc) as tc:
        with tc.tile_pool(name="const", bufs=1) as const, \
             tc.tile_pool(name="work", bufs=3) as work, \
             tc.tile_pool(name="ot", bufs=2) as otp, \
             tc.tile_pool(name="ste", bufs=6) as ste_pool, \
             tc.tile_pool(name="psA", bufs=2, space="PSUM") as psA, \
             tc.tile_pool(name="psB", bufs=2, space="PSUM") as psB:

            pools = {"work": work, "psA": psA, "psB": psB,
                     "ste": ste_pool, "ot": otp}

            C = {}
            for wname in ("wqT", "wkT", "wvT", "woT"):
                C[wname] = const.tile([P, DT, D], BF16, name=wname)
                nc.sync.dma_start(
                    out=C[wname][:, :, :],
                    in_=di[wname][:].rearrange("(d p) e -> p d e", p=P))
            for wname in ("bq_col", "bk_col", "bv_bc"):
                C[wname] = const.tile(list(di[wname].shape), F32, name=wname)
                nc.sync.dma_start(out=C[wname][:], in_=di[wname][:])
            C["bo_row"] = const.tile([1, D], F32R, name="bo_row")
            nc.sync.dma_start(out=C["bo_row"][:], in_=di["bo_row"][:])
            for wname in ("ra0_bc", "rb0_bc", "ra1_bc", "rb1_bc",
                          "a0_bc", "b0_bc"):
                C[wname] = const.tile([P, D], F32, name=wname)
                nc.sync.dma_start(out=C[wname][:, :], in_=di[wname][:])
            C["ones"] = const.tile([1, P], F32R, name="ones")
            nc.sync.dma_start(out=C["ones"][:, :], in_=di["ones_in"][:])
            C["ident"] = const.tile([P, P], BF16, name="ident")
            nc.sync.dma_start(out=C["ident"][:, :], in_=di["ident_in"][:])

            C["zT"] = const.tile([P, DT, S], BF16, name="zT")
            C["qt"] = const.tile([P, DT, S], BF16, name="qt")
            C["kt"] = const.tile([P, DT, S], BF16, name="kt")
            C["v_aug"] = const.tile([P, NTS, VROW], BF16, name="v_aug")
            nc.vector.memset(C["v_aug"][:, :, :], 1.0)

            C1 = dict(C)
            C1["ra_bc"], C1["rb_bc"] = C["ra0_bc"], C["rb0_bc"]
            _build_block(nc, pools, C1, di["xs"][:], x2_d[:], S, 0)
            C2 = dict(C)
            C2["ra_bc"], C2["rb_bc"] = C["ra1_bc"], C["rb1_bc"]
            _build_block(nc, pools, C2, x2_d[:], out_d[:], OWN, 1)

    _fix_sync_waits(nc)
    return nc


_NC_CACHE = None


def _get_nc():
    global _NC_CACHE
    if _NC_CACHE is None:
        _NC_CACHE = _build_program()
    return _NC_CACHE


def _prep_inputs(x, a0, b0, ra0, rb0, ra1, rb1,
                 wq, bq, wk, bk, wv, bv, wo, bo):
    bf = ml_dtypes.bfloat16
    base = {
        "wqT": np.ascontiguousarray(np.asarray(wq, np.float32).T).astype(bf),
        "wkT": np.ascontiguousarray(np.asarray(wk, np.float32).T).astype(bf),
        "wvT": np.ascontiguousarray(np.asarray(wv, np.float32).T).astype(bf),
        "woT": np.ascontiguousarray(np.asarray(wo, np.float32).T).astype(bf),
        "bq_col": np.ascontiguousarray(
            np.asarray(bq, np.float32).reshape(DT, P).T),
        "bk_col": np.ascontiguousarray(
            np.asarray(bk, np.float32).reshape(DT, P).T),
        "bv_bc": np.ascontiguousarray(
            np.broadcast_to(np.asarray(bv, np.float32), (P, D))),
        "bo_row": np.asarray(bo, np.float32).reshape(1, D).copy(),
        "ra0_bc": np.ascontiguousarray(
            np.broadcast_to(np.asarray(ra0, np.float32), (P, D))),
        "rb0_bc": np.ascontiguousarray(
            np.broadcast_to(np.asarray(rb0, np.float32), (P, D))),
        "ra1_bc": np.ascontiguousarray(
            np.broadcast_to(np.asarray(ra1, np.float32), (P, D))),
        "rb1_bc": np.ascontiguousarray(
            np.broadcast_to(np.asarray(rb1, np.float32), (P, D))),
        "a0_bc": np.ascontiguousarray(
            np.broadcast_to(np.asarray(a0, np.float32), (P, D))),
        "b0_bc": np.ascontiguousarray(
            np.broadcast_to(np.asarray(b0, np.float32), (P, D))),
        "ones_in": np.ones((1, P), np.float32),
        "ident_in": np.eye(P, dtype=np.float32).astype(bf),
    }
    x = np.asarray(x, np.float32)
    in_maps = []
    for c in range(8):
        b, q0 = c // GRP, (c % GRP) * OWN
        m = dict(base)
        # rotate tokens so this core's output shard sits at rows 0..OWN
        m["xs"] = np.ascontiguousarray(
            np.concatenate([x[b, q0:], x[b, :q0]], axis=0))
        in_maps.append(m)
    return in_maps


def kernel(**inputs):
    nc = _get_nc()
    in_maps = _prep_inputs(**inputs)
    res = run_bass_kernel_spmd(nc, in_maps, list(range(8)))
    B = inputs["x"].shape[0]
    out = np.empty((B, S, D), np.float32)
    for c in range(8):
        b, q0 = c // GRP, (c % GRP) * OWN
        out[b, q0:q0 + OWN, :] = res.results[c]["out"]
    return out


if __name__ == "__main__":
    rng = np.random.default_rng(0)
    ins = {
        "x": rng.standard_normal((2, S, D)).astype(np.float32),
        "a0": np.ones(D, np.float32), "b0": np.zeros(D, np.float32),
        "ra0": np.ones(D, np.float32), "rb0": np.zeros(D, np.float32),
        "ra1": np.ones(D, np.float32), "rb1": np.zeros(D, np.float32),
        "wq": (rng.standard_normal((D, D)) * 0.02).astype(np.float32),
        "bq": np.zeros(D, np.float32),
        "wk": (rng.standard_normal((D, D)) * 0.02).astype(np.float32),
        "bk": np.zeros(D, np.float32),
        "wv": (rng.standard_normal((D, D)) * 0.02).astype(np.float32),
        "bv": np.zeros(D, np.float32),
        "wo": (rng.standard_normal((D, D)) * 0.02).astype(np.float32),
        "bo": np.zeros(D, np.float32),
    }
    out = kernel(**ins)
    print("kernel ran, out shape", out.shape, out.dtype)
